# revision 4
# baseline (speedup 1.0000x reference)
"""Trainium2 Bass kernel for nn_CanonicalColorLoss (masked per-part chamfer color loss).

Strategy
--------
For each object o (first 15 of 16) and part m, the reference computes a
masked chamfer distance between pred/true color point sets restricted to
the SAME mask, so we gather each part's points host-side (n ~ 1536) and
compute, per (o, m, direction), the full n x n squared-distance matrix on
the TensorEngine as a K=11 bf16 hi/lo matmul:

    d2[x, y] = q[x] + ( -2 p[x].t[y] + r[y] )
               \______ matmul output in PSUM ______/

where p is split p = ph + pl (bf16 hi/lo; products of bf16 pairs are exact
in fp32, only the pl*tl term is dropped -> ~1e-4 abs error on d2), and
q/r are the squared norms (r rides in the matmul split hi/lo, q is added
on the host after the min since min_y(d2) = q + min_y(-2pt + r)).

The VectorEngine min-reduces each PSUM row-tile over its free dim, giving
min_y per point. sqrt + masked means + the final scalar reduction over a
few thousand values happen on the host in exact fp32.

The 240 (o, m, dir) units are sorted by n and dealt round-robin to the 8
cores (30 units each, SPMD: one program, per-core data). Slot widths are
shared across cores (max over the 8 units in the slot, padded to 128).

Raw bass (not Tile): this toolchain's walrus rejects instructions with
more than one semaphore wait, so sync is hand-rolled: standalone wait_ge
instructions plus per-slot DMA semaphores (HWDGE completions are not
ordered across transfers, so each in-flight buffer slot gets its own
semaphore).
"""
import numpy as np
import ml_dtypes

import concourse.bass as bass
import concourse.mybir as mybir
from concourse.bass_utils import run_bass_kernel_spmd

B, M, P = 16, 8, 3072
NB = B - 1          # reference skips the last object
N_CORES = 8
FAR = 1.0e4         # padded columns sit ~3e8 away in d2; never the min
PSUM_COLS = 2048    # one double-buffer half: 4 banks of 512 fp32
K = 15              # matmul contraction rows (full hi/lo split + 3-way r)

f32 = mybir.dt.float32
bf16 = mybir.dt.bfloat16


def _bf(x):
    return x.astype(ml_dtypes.bfloat16).astype(np.float32)


def _split(x):
    h = _bf(x)
    return h, _bf(x - h)


def _prepare_units(canoncolor_out, gt_color, pt_offset, mask_pts):
    """Gather per-(object, part) masked point sets; emit 2 directions each."""
    N = canoncolor_out.shape[0]
    starts = np.concatenate([np.zeros(1, np.int64),
                             pt_offset.astype(np.int64)[:-1]])
    idx = np.clip(starts[:NB, None] + np.arange(P, dtype=np.int64), 0, N - 1)
    pred = np.ascontiguousarray(canoncolor_out[idx])  # [NB, P, 3]
    true = np.ascontiguousarray(gt_color[idx])
    units = []  # (o, m, dirn, n, rows_pts, cols_pts)
    for o in range(NB):
        for m in range(M):
            msk = mask_pts[o, m]
            n = int(msk.sum())
            pr = pred[o][msk]
            tr = true[o][msk]
            units.append((o, m, 0, n, pr, tr))  # rows=pred, cols=true
            units.append((o, m, 1, n, tr, pr))  # rows=true, cols=pred
    return units


def _build_unit_arrays(rows, cols, n, W):
    """lhsT [K, W] bf16, rhs [K, W] bf16, q [W] f32 for one unit.

    Rows (points whose nearest neighbour we seek) pad with zeros (host
    ignores padded rows). Columns pad with FAR so they never win the min.
    """
    rows_p = np.zeros((W, 3), np.float32)
    rows_p[:n] = rows
    cols_p = np.full((W, 3), FAR, np.float32)
    cols_p[:n] = cols
    q = (rows_p ** 2).sum(1, dtype=np.float32)
    r = (cols_p ** 2).sum(1, dtype=np.float32)
    ph, pl = _split(-2.0 * rows_p)   # [W, 3]
    th, tl = _split(cols_p)
    r0 = _bf(r)
    r1 = _bf(r - r0)
    r2 = _bf(r - r0 - r1)
    ones = np.ones(W, np.float32)
    lhsT = np.stack([ph[:, 0], ph[:, 1], ph[:, 2],
                     pl[:, 0], pl[:, 1], pl[:, 2],
                     ph[:, 0], ph[:, 1], ph[:, 2],
                     pl[:, 0], pl[:, 1], pl[:, 2],
                     ones, ones, ones])                # [K, W]
    rhs = np.stack([th[:, 0], th[:, 1], th[:, 2],
                    th[:, 0], th[:, 1], th[:, 2],
                    tl[:, 0], tl[:, 1], tl[:, 2],
                    tl[:, 0], tl[:, 1], tl[:, 2],
                    r0, r1, r2])                       # [K, W]
    return (lhsT.astype(ml_dtypes.bfloat16),
            rhs.astype(ml_dtypes.bfloat16), q)


def _build_kernel(slot_plan, n_cols):
    """slot_plan: list of (W, rtiles, groups) per slot; n_cols = total
    minbuf columns = sum over slots of rtiles * groups."""
    nc = bass.Bass()
    n_slots = len(slot_plan)
    slab_cols = [2 * W for (W, _, _) in slot_plan]          # lhsT | rhs
    slab_off = np.concatenate([[0], np.cumsum(slab_cols)]).astype(int)
    total_slab = int(slab_off[-1])
    max_slab = max(slab_cols)

    data_d = nc.dram_tensor("data", [K, total_slab], bf16, kind="ExternalInput")
    out_d = nc.dram_tensor("minbuf", [128, n_cols], f32, kind="ExternalOutput")

    n_bufs = 3
    with (
        nc.semaphore("s_slot0") as s0,
        nc.semaphore("s_slot1") as s1,
        nc.semaphore("s_slot2") as s2,
        nc.semaphore("mm_sem") as mm_sem,
        nc.semaphore("red_sem") as red_sem,
        nc.semaphore("peu_sem") as peu_sem,
        nc.semaphore("out_sem") as out_sem,
        nc.sbuf_tensor("slab0", [K, max_slab], bf16) as slab0,
        nc.sbuf_tensor("slab1", [K, max_slab], bf16) as slab1,
        nc.sbuf_tensor("slab2", [K, max_slab], bf16) as slab2,
        nc.sbuf_tensor("warm", [K, 128], bf16) as dummy,
        nc.sbuf_tensor("minsb", [128, n_cols], f32) as minbuf,
        nc.psum_tensor("ps0", [128, PSUM_COLS], f32) as ps0,
        nc.psum_tensor("ps1", [128, PSUM_COLS], f32) as ps1,
    ):
        slot_sems = [s0, s1, s2]
        slabs = [slab0, slab1, slab2]
        psb = [ps0, ps1]

        # per-slot tile schedule: global tile index and minbuf column map
        # tile entry: (slot u, row-tile t, group g, W of group, col index)
        tiles = []
        col = 0
        for u, (W, rt, gr) in enumerate(slot_plan):
            for t in range(rt):
                for g in range(gr):
                    gw = min(PSUM_COLS, W - g * PSUM_COLS)
                    tiles.append((u, t, g, gw, col))
                    col += 1
        assert col == n_cols

        with nc.Block() as block:

            @block.sync
            def _(sync):
                for u in range(n_slots):
                    if u >= n_bufs:
                        sync.wait_ge(peu_sem, u - (n_bufs - 1))
                    sync.dma_start(
                        slabs[u % n_bufs][:, 0:slab_cols[u]],
                        data_d[:, int(slab_off[u]):int(slab_off[u + 1])],
                    ).then_inc(slot_sems[u % n_bufs], 16)
                sync.wait_ge(red_sem, len(tiles))
                sync.dma_start(out_d[:], minbuf[:]).then_inc(out_sem, 16)
                sync.wait_ge(out_sem, 16)

            @block.tensor
            def _(tensor):
                # flush PE pipeline state (first matmul after the axon
                # preamble has been observed corrupted on core 0)
                for _ in range(2):
                    tensor.matmul(ps0[:, 0:128], dummy[:, 0:128],
                                  dummy[:, 0:128], start=True, stop=True)
                gi = 0
                for u, (W, rt, gr) in enumerate(slot_plan):
                    buf = slabs[u % n_bufs]
                    tensor.wait_ge(slot_sems[u % n_bufs],
                                   16 * (u // n_bufs + 1))
                    for t in range(rt):
                        for g in range(gr):
                            gw = min(PSUM_COLS, W - g * PSUM_COLS)
                            ps = psb[gi % 2]
                            if gi >= 2:
                                tensor.wait_ge(red_sem, gi - 1)
                            lt = buf[:, t * 128:(t + 1) * 128]
                            c0 = g * PSUM_COLS
                            mm = None
                            for cc in range(0, gw, 512):
                                cw = min(512, gw - cc)
                                mm = tensor.matmul(
                                    ps[:, cc:cc + cw],
                                    lt,
                                    buf[:, W + c0 + cc:W + c0 + cc + cw],
                                    start=True, stop=True)
                            mm.then_inc(mm_sem, 1)
                            gi += 1
                    tensor.nop().then_inc(peu_sem, 1)

            @block.vector
            def _(vector):
                for gi, (u, t, g, gw, c) in enumerate(tiles):
                    vector.wait_ge(mm_sem, gi + 1)
                    vector.tensor_reduce(
                        out=minbuf[:, c:c + 1],
                        in_=psb[gi % 2][:, 0:gw],
                        axis=mybir.AxisListType.X,
                        op=mybir.AluOpType.min,
                    ).then_inc(red_sem, 1)

    return nc, tiles


def kernel(canoncolor_out, gt_color, pt_offset, mask_pts):
    canoncolor_out = np.asarray(canoncolor_out, dtype=np.float32)
    gt_color = np.asarray(gt_color, dtype=np.float32)
    pt_offset = np.asarray(pt_offset)
    mask_pts = np.asarray(mask_pts)

    units = _prepare_units(canoncolor_out, gt_color, pt_offset, mask_pts)
    n_units = len(units)  # 240
    order = sorted(range(n_units), key=lambda i: -units[i][3])
    n_slots_all = (n_units + N_CORES - 1) // N_CORES  # 30

    # slot_plan shared across cores
    slot_plan = []
    slot_units = []  # [slot][core] -> unit index or None
    for s in range(n_slots_all):
        grp = order[s * N_CORES:(s + 1) * N_CORES]
        maxn = max(units[i][3] for i in grp)
        if maxn == 0:
            continue  # all-empty parts: nothing to compute on device
        W = max(128, -(-maxn // 128) * 128)
        rt = W // 128
        gr = -(-W // PSUM_COLS)
        slot_plan.append((W, rt, gr))
        row = [None] * N_CORES
        for c, i in enumerate(grp):
            row[c] = i
        slot_units.append(row)

    n_cols = sum(rt * gr for (_, rt, gr) in slot_plan)

    sums = np.zeros((NB, M, 2), np.float32)
    ns = np.zeros((NB, M), np.int64)
    for (o, m, dirn, n, _, _) in units:
        ns[o, m] = n

    if slot_plan:
        nc, tiles = _build_kernel(slot_plan, n_cols)

        # build per-core inputs
        in_maps = []
        qs = [[] for _ in range(N_CORES)]  # per core per slot: q vector
        for c in range(N_CORES):
            parts = []
            for s, (W, rt, gr) in enumerate(slot_plan):
                i = slot_units[s][c]
                if i is None:
                    lhsT = np.zeros((K, W), ml_dtypes.bfloat16)
                    rhs = np.full((K, W), 1.0, ml_dtypes.bfloat16)
                    q = np.zeros(W, np.float32)
                else:
                    (_, _, _, n, rows, cols) = units[i]
                    lhsT, rhs, q = _build_unit_arrays(rows, cols, n, W)
                parts.append(np.concatenate([lhsT, rhs], axis=1))
                qs[c].append(q)
            in_maps.append({"data": np.concatenate(parts, axis=1)})

        res = run_bass_kernel_spmd(nc, in_maps, core_ids=list(range(N_CORES)))

        # host finalize: per unit, min across groups, +q, clamp, sqrt, sum
        # minbuf column layout: for slot u: rt*gr columns in (t, g) order
        col_of = {}
        for (u, t, g, gw, c) in tiles:
            col_of[(u, t, g)] = c
        for c in range(N_CORES):
            mb = res.results[c]["minbuf"]  # [128, n_cols]
            for s, (W, rt, gr) in enumerate(slot_plan):
                i = slot_units[s][c]
                if i is None:
                    continue
                (o, m, dirn, n, _, _) = units[i]
                if n == 0:
                    continue
                # rows r of tile t live at partition r-128t, column (s,t,g)
                mins = np.empty((rt, 128), np.float32)
                mins.fill(np.inf)
                for t in range(rt):
                    gcols = np.stack([mb[:, col_of[(s, t, g)]]
                                      for g in range(gr)])
                    mins[t] = gcols.min(axis=0)
                flat = mins.reshape(-1)[:n]
                d2 = np.maximum(flat + qs[c][s][:n], 0.0)
                sums[o, m, dirn] = np.sqrt(d2).sum(dtype=np.float32)

    # final scalar math, mirroring the reference in fp32
    nf = ns.astype(np.float32)
    denom = np.maximum(nf, 1.0).astype(np.float32)
    mean_x = sums[:, :, 0] / denom
    mean_y = sums[:, :, 1] / denom
    ch = (mean_x + mean_y) * np.float32(0.5)
    valid = ns >= 2
    nvalid = valid.sum(axis=1)
    obj_loss = np.where(
        nvalid > 0,
        (ch * valid).sum(axis=1, dtype=np.float32)
        / np.maximum(nvalid, 1).astype(np.float32),
        np.float32(0.0),
    ).astype(np.float32)
    counted = nvalid > 0
    count = int(counted.sum())
    total = np.float32((obj_loss * counted).sum(dtype=np.float32))
    if count > 0:
        out = np.float32(total / np.float32(count))
    else:
        out = np.float32(0.0)
    return np.asarray(out, dtype=np.float32)


# revision 5
# speedup vs baseline: 590.8772x; 590.8772x over previous
r"""Trainium2 Bass kernel for nn_CanonicalColorLoss (masked per-part chamfer color loss).

Strategy
--------
For each object o (first 15 of 16) and part m, the reference computes a
masked chamfer distance between pred/true color point sets restricted to
the SAME mask, so we gather each part's points host-side (n ~ 1536) and
compute, per (o, m, direction), the full n x n squared-distance matrix on
the TensorEngine as a K=14 fp16 hi/lo matmul:

    d2[x, y] = q[x] + ( -2 p[x].t[y] + r[y] )   <- matmul output in PSUM

p and t are split 2-way into fp16 hi+lo (products of fp16 pairs are exact
in fp32; all four cross products are kept, so the only error is the
~2^-22 representation residual), r = |t|^2 rides along as two fp16 rows,
and q = |p|^2 is added on the host after the min, exactly, since
min_y(d2) = q + min_y(-2pt + r).

The VectorEngine min-reduces each PSUM row-tile over its free dim, giving
min_y per point. sqrt + masked means + the final scalar reduction over a
few thousand values happen on the host in exact fp32.

The 240 (o, m, dir) units are sorted by n and dealt round-robin to the 8
cores (30 units each, SPMD: one program, per-core data). Slot widths are
shared across cores (max over the 8 units in the slot, padded to 128).

Raw bass (not Tile): this toolchain's walrus rejects instructions with
more than one semaphore wait, so sync is hand-rolled: standalone wait_ge
instructions plus per-slot DMA semaphores (HWDGE completions are not
ordered across transfers, so each in-flight buffer slot gets its own
semaphore).
"""
import numpy as np
import ml_dtypes

import concourse.bass as bass
import concourse.mybir as mybir
from concourse.bass_utils import run_bass_kernel_spmd

B, M, P = 16, 8, 3072
NB = B - 1          # reference skips the last object
N_CORES = 8
PSUM_COLS = 2048    # one double-buffer half: 4 banks of 512 fp32

f32 = mybir.dt.float32


def _prepare_units(canoncolor_out, gt_color, pt_offset, mask_pts):
    """Gather per-(object, part) masked point sets; emit 2 directions each."""
    N = canoncolor_out.shape[0]
    starts = np.concatenate([np.zeros(1, np.int64),
                             pt_offset.astype(np.int64)[:-1]])
    idx = np.clip(starts[:NB, None] + np.arange(P, dtype=np.int64), 0, N - 1)
    pred = np.ascontiguousarray(canoncolor_out[idx])  # [NB, P, 3]
    true = np.ascontiguousarray(gt_color[idx])
    units = []  # (o, m, dirn, n, rows_pts, cols_pts)
    for o in range(NB):
        for m in range(M):
            msk = mask_pts[o, m]
            n = int(msk.sum())
            pr = pred[o][msk]
            tr = true[o][msk]
            units.append((o, m, 0, n, pr, tr))  # rows=pred, cols=true
            units.append((o, m, 1, n, tr, pr))  # rows=true, cols=pred
    return units


class Layout:
    """Operand layout: fp16 2-way split (preferred) or bf16 3-way split
    (fallback when values exceed the fp16-safe range)."""

    def __init__(self, max_abs):
        self.use_fp16 = max_abs <= 35.0
        if self.use_fp16:
            self.far = 140.0
            self.k_rows = 14
            self.mdt = mybir.dt.float16
            self.npdt = np.float16
        else:
            self.far = 1.0e4
            self.k_rows = 21
            self.mdt = mybir.dt.bfloat16
            self.npdt = ml_dtypes.bfloat16

    def _cast(self, x):
        return x.astype(self.npdt).astype(np.float32)

    def build_unit(self, rows, cols, n, W):
        """lhsT [K, W], rhs [K, W] (self.npdt), q [W] f32 for one unit.

        Rows (points whose nearest neighbour we seek) pad with zeros (the
        host ignores padded rows). Columns pad with far so they never win
        the min.
        """
        rows_p = np.zeros((W, 3), np.float32)
        rows_p[:n] = rows
        cols_p = np.full((W, 3), self.far, np.float32)
        cols_p[:n] = cols
        q = (rows_p ** 2).sum(1, dtype=np.float32)
        r = (cols_p ** 2).sum(1, dtype=np.float32)
        p = -2.0 * rows_p
        t = cols_p
        ones = np.ones(W, np.float32)
        if self.use_fp16:
            ph = self._cast(p)
            pl = self._cast(p - ph)
            th = self._cast(t)
            tl = self._cast(t - th)
            r0 = self._cast(r)
            r1 = self._cast(r - r0)
            lhsT = np.stack([ph[:, 0], ph[:, 1], ph[:, 2],
                             pl[:, 0], pl[:, 1], pl[:, 2],
                             ph[:, 0], ph[:, 1], ph[:, 2],
                             pl[:, 0], pl[:, 1], pl[:, 2],
                             ones, ones])
            rhs = np.stack([th[:, 0], th[:, 1], th[:, 2],
                            th[:, 0], th[:, 1], th[:, 2],
                            tl[:, 0], tl[:, 1], tl[:, 2],
                            tl[:, 0], tl[:, 1], tl[:, 2],
                            r0, r1])
        else:
            # 3-way bf16 splits; keep the 6 largest product terms
            p0 = self._cast(p)
            p1 = self._cast(p - p0)
            p2 = self._cast(p - p0 - p1)
            t0 = self._cast(t)
            t1 = self._cast(t - t0)
            t2 = self._cast(t - t0 - t1)
            r0 = self._cast(r)
            r1 = self._cast(r - r0)
            r2 = self._cast(r - r0 - r1)
            stacks_l, stacks_r = [], []
            for (a, b) in [(p0, t0), (p0, t1), (p1, t0),
                           (p0, t2), (p1, t1), (p2, t0)]:
                for cmp_ in range(3):
                    stacks_l.append(a[:, cmp_])
                    stacks_r.append(b[:, cmp_])
            stacks_l += [ones, ones, ones]
            stacks_r += [r0, r1, r2]
            lhsT = np.stack(stacks_l)
            rhs = np.stack(stacks_r)
        return lhsT.astype(self.npdt), rhs.astype(self.npdt), q


def _build_kernel(slot_plan, n_cols, layout, repeat=1):
    """slot_plan: list of (W, rtiles, groups) per slot; n_cols = total
    minbuf columns = sum over slots of rtiles * groups. repeat re-runs the
    whole schedule (same data/columns) for timing regression."""
    nc = bass.Bass()
    n_slots = len(slot_plan)
    Kr = layout.k_rows
    slab_cols = [2 * W for (W, _, _) in slot_plan]          # lhsT | rhs
    slab_off = np.concatenate([[0], np.cumsum(slab_cols)]).astype(int)
    total_slab = int(slab_off[-1])
    max_slab = max(slab_cols)

    data_d = nc.dram_tensor("data", [Kr, total_slab], layout.mdt,
                            kind="ExternalInput")
    out_d = nc.dram_tensor("minbuf", [128, n_cols], f32, kind="ExternalOutput")

    # slot column bases in minbuf
    col_base = []
    col = 0
    for (W, rt, gr) in slot_plan:
        col_base.append(col)
        col += rt * gr
    assert col == n_cols

    n_bufs = 3
    n_units_total = n_slots * repeat
    with (
        nc.semaphore("s_slot0") as s0,
        nc.semaphore("s_slot1") as s1,
        nc.semaphore("s_slot2") as s2,
        nc.semaphore("mm_sem") as mm_sem,
        nc.semaphore("red_sem") as red_sem,
        nc.semaphore("peu_sem") as peu_sem,
        nc.semaphore("out_sem") as out_sem,
        nc.sbuf_tensor("slab0", [Kr, max_slab], layout.mdt) as slab0,
        nc.sbuf_tensor("slab1", [Kr, max_slab], layout.mdt) as slab1,
        nc.sbuf_tensor("slab2", [Kr, max_slab], layout.mdt) as slab2,
        nc.sbuf_tensor("warm", [Kr, 128], layout.mdt) as dummy,
        nc.sbuf_tensor("minsb", [128, n_cols], f32) as minbuf,
        nc.psum_tensor("ps0", [128, PSUM_COLS], f32) as ps0,
        nc.psum_tensor("ps1", [128, PSUM_COLS], f32) as ps1,
    ):
        slot_sems = [s0, s1, s2]
        slabs = [slab0, slab1, slab2]
        psb = [ps0, ps1]

        # global tile schedule: (slot s, row-tile t, group g, width, col)
        tiles = []
        for rep in range(repeat):
            for s, (W, rt, gr) in enumerate(slot_plan):
                for t in range(rt):
                    for g in range(gr):
                        gw = min(PSUM_COLS, W - g * PSUM_COLS)
                        tiles.append((s, t, g, gw, col_base[s] + t * gr + g))

        with nc.Block() as block:

            @block.sync
            def _(sync):
                for u in range(n_units_total):
                    s = u % n_slots
                    if u >= n_bufs:
                        sync.wait_ge(peu_sem, u - (n_bufs - 1))
                    sync.dma_start(
                        slabs[u % n_bufs][:, 0:slab_cols[s]],
                        data_d[:, int(slab_off[s]):int(slab_off[s + 1])],
                    ).then_inc(slot_sems[u % n_bufs], 16)
                sync.wait_ge(red_sem, len(tiles))
                sync.dma_start(out_d[:], minbuf[:]).then_inc(out_sem, 16)
                sync.wait_ge(out_sem, 16)

            @block.tensor
            def _(tensor):
                # flush PE pipeline state (first matmul after the axon
                # preamble has been observed corrupted on core 0)
                for _ in range(2):
                    tensor.matmul(ps0[:, 0:128], dummy[:, 0:128],
                                  dummy[:, 0:128], start=True, stop=True)
                gi = 0
                for u in range(n_units_total):
                    s = u % n_slots
                    (W, rt, gr) = slot_plan[s]
                    buf = slabs[u % n_bufs]
                    tensor.wait_ge(slot_sems[u % n_bufs],
                                   16 * (u // n_bufs + 1))
                    for t in range(rt):
                        for g in range(gr):
                            gw = min(PSUM_COLS, W - g * PSUM_COLS)
                            ps = psb[gi % 2]
                            if gi >= 2:
                                tensor.wait_ge(red_sem, gi - 1)
                            lt = buf[:, t * 128:(t + 1) * 128]
                            c0 = g * PSUM_COLS
                            mm = None
                            for cc in range(0, gw, 512):
                                cw = min(512, gw - cc)
                                mm = tensor.matmul(
                                    ps[:, cc:cc + cw],
                                    lt,
                                    buf[:, W + c0 + cc:W + c0 + cc + cw],
                                    start=True, stop=True)
                            mm.then_inc(mm_sem, 1)
                            gi += 1
                    tensor.nop().then_inc(peu_sem, 1)

            @block.vector
            def _(vector):
                for gi, (s, t, g, gw, c) in enumerate(tiles):
                    vector.wait_ge(mm_sem, gi + 1)
                    vector.tensor_reduce(
                        out=minbuf[:, c:c + 1],
                        in_=psb[gi % 2][:, 0:gw],
                        axis=mybir.AxisListType.X,
                        op=mybir.AluOpType.min,
                    ).then_inc(red_sem, 1)

    return nc


def _plan(units):
    """Balanced slot assignment shared across cores."""
    order = sorted(range(len(units)), key=lambda i: -units[i][3])
    n_slots_all = (len(units) + N_CORES - 1) // N_CORES
    slot_plan = []
    slot_units = []
    for s in range(n_slots_all):
        grp = order[s * N_CORES:(s + 1) * N_CORES]
        maxn = max(units[i][3] for i in grp)
        if maxn == 0:
            continue
        W = max(128, -(-maxn // 128) * 128)
        slot_plan.append((W, W // 128, -(-W // PSUM_COLS)))
        row = [None] * N_CORES
        for c, i in enumerate(grp):
            row[c] = i
        slot_units.append(row)
    return slot_plan, slot_units


def _core_inputs(units, slot_plan, slot_units, layout):
    in_maps = []
    qs = [[] for _ in range(N_CORES)]
    for c in range(N_CORES):
        parts = []
        for s, (W, rt, gr) in enumerate(slot_plan):
            i = slot_units[s][c]
            if i is None:
                lhsT = np.zeros((layout.k_rows, W), layout.npdt)
                rhs = np.full((layout.k_rows, W), 1.0, layout.npdt)
                q = np.zeros(W, np.float32)
            else:
                (_, _, _, n, rows, cols) = units[i]
                lhsT, rhs, q = layout.build_unit(rows, cols, n, W)
            parts.append(np.concatenate([lhsT, rhs], axis=1))
            qs[c].append(q)
        in_maps.append({"data": np.ascontiguousarray(
            np.concatenate(parts, axis=1))})
    return in_maps, qs


def kernel(canoncolor_out, gt_color, pt_offset, mask_pts):
    canoncolor_out = np.asarray(canoncolor_out, dtype=np.float32)
    gt_color = np.asarray(gt_color, dtype=np.float32)
    pt_offset = np.asarray(pt_offset)
    mask_pts = np.asarray(mask_pts)

    units = _prepare_units(canoncolor_out, gt_color, pt_offset, mask_pts)
    max_abs = max(float(np.abs(canoncolor_out).max() if canoncolor_out.size else 0.0),
                  float(np.abs(gt_color).max() if gt_color.size else 0.0))
    layout = Layout(max_abs)
    slot_plan, slot_units = _plan(units)
    n_cols = sum(rt * gr for (_, rt, gr) in slot_plan)

    sums = np.zeros((NB, M, 2), np.float32)
    ns = np.zeros((NB, M), np.int64)
    for (o, m, dirn, n, _, _) in units:
        ns[o, m] = n

    if slot_plan:
        nc = _build_kernel(slot_plan, n_cols, layout)
        in_maps, qs = _core_inputs(units, slot_plan, slot_units, layout)
        res = run_bass_kernel_spmd(nc, in_maps, core_ids=list(range(N_CORES)))

        col_base = np.concatenate(
            [[0], np.cumsum([rt * gr for (_, rt, gr) in slot_plan])]).astype(int)
        for c in range(N_CORES):
            mb = res.results[c]["minbuf"]  # [128, n_cols]
            for s, (W, rt, gr) in enumerate(slot_plan):
                i = slot_units[s][c]
                if i is None:
                    continue
                (o, m, dirn, n, _, _) = units[i]
                if n == 0:
                    continue
                cols_s = mb[:, col_base[s]:col_base[s] + rt * gr]
                # columns are (t, g); min across groups then unravel tiles
                mins = cols_s.reshape(128, rt, gr).min(axis=2)  # [128, rt]
                flat = mins.T.reshape(-1)[:n]
                d2 = np.maximum(flat + qs[c][s][:n], 0.0)
                sums[o, m, dirn] = np.sqrt(d2).sum(dtype=np.float32)

    # final scalar math, mirroring the reference in fp32
    nf = ns.astype(np.float32)
    denom = np.maximum(nf, 1.0).astype(np.float32)
    mean_x = sums[:, :, 0] / denom
    mean_y = sums[:, :, 1] / denom
    ch = (mean_x + mean_y) * np.float32(0.5)
    valid = ns >= 2
    nvalid = valid.sum(axis=1)
    obj_loss = np.where(
        nvalid > 0,
        (ch * valid).sum(axis=1, dtype=np.float32)
        / np.maximum(nvalid, 1).astype(np.float32),
        np.float32(0.0),
    ).astype(np.float32)
    counted = nvalid > 0
    count = int(counted.sum())
    total = np.float32((obj_loss * counted).sum(dtype=np.float32))
    if count > 0:
        out = np.float32(total / np.float32(count))
    else:
        out = np.float32(0.0)
    return np.asarray(out, dtype=np.float32)
